# revision 5
# baseline (speedup 1.0000x reference)
"""Trainium2 Bass kernel for a 6-layer RealNVP-style conditional flow.

kernel(x, context, W1, b1, W2, b2) -> (x_out [65536,256] f32, log_det [65536] f32)

Strategy:
  - Pure data parallelism: batch sharded 8192 rows/core across 8 NeuronCores,
    weights replicated.
  - Activations kept transposed [feature, batch] in SBUF; x stored as
    de-interleaved even/odd feature masters so each coupling layer's
    masked/unmasked halves are contiguous 128-partition tiles.
  - All matmuls in split-bf16 3-product form (Xh@Wh + Xl@Wh + Xh@Wl with
    fp32 PSUM accumulation) for ~22-bit effective mantissa at 1 cycle/row.
    Weights and context are split on the host; x and h are split on device.
  - log_det accumulated as sum of tanh(st) in SBUF, folded over features by a
    single [128,1] fives-vector matmul per 512-column tile (x5 scale baked in).
"""

import os
import sys
from contextlib import ExitStack

import numpy as np

for _p in ('/opt/trn_rl_repo', '/root/.axon_site/_ro/trn_rl_repo'):
    if os.path.isdir(_p) and _p not in sys.path:
        sys.path.insert(0, _p)

import ml_dtypes  # noqa: E402
import concourse.bass as bass  # noqa: E402
import concourse.tile as tile  # noqa: E402
from concourse import bacc, mybir  # noqa: E402
from concourse.bass import ds  # noqa: E402
from concourse.masks import make_identity  # noqa: E402

dt = mybir.dt
F32 = dt.float32
F32R = dt.float32r
BF16 = dt.bfloat16
AF = mybir.ActivationFunctionType
ALU = mybir.AluOpType

NCORES = 8
B = 65536
RB = B // NCORES      # 8192 rows per core
DIM = 256
HALF = 128
CTXF = 256
HID = 1024
NL = 6
S_MAX = 5.0

NB = 4096             # batch columns per block (2 blocks per core)
NT = 512              # batch-tile columns (matmul moving dim)
NBLK = RB // NB       # 2
T_PER_B = NB // NT    # 8
C128 = NB // 128      # 32 column-tiles of 128 for transposes


def _build_program(b1_zero=True):
    nc = bacc.Bacc('TRN2', target_bir_lowering=False, debug=False,
                   num_devices=NCORES)

    x_d = nc.dram_tensor('x', [RB, DIM], F32, kind='ExternalInput')
    ch_d = nc.dram_tensor('ctxh', [RB, CTXF], BF16, kind='ExternalInput')
    cl_d = nc.dram_tensor('ctxl', [RB, CTXF], BF16, kind='ExternalInput')
    w1h_d = nc.dram_tensor('w1h', [NL, HALF + CTXF, HID], BF16, kind='ExternalInput')
    w1l_d = nc.dram_tensor('w1l', [NL, HALF + CTXF, HID], BF16, kind='ExternalInput')
    w2h_d = nc.dram_tensor('w2h', [NL, HID, 2 * HALF], BF16, kind='ExternalInput')
    w2l_d = nc.dram_tensor('w2l', [NL, HID, 2 * HALF], BF16, kind='ExternalInput')
    b1_d = nc.dram_tensor('b1', [NL, HID], F32, kind='ExternalInput')
    b2_d = nc.dram_tensor('b2', [NL, 2 * HALF], F32, kind='ExternalInput')
    xo_d = nc.dram_tensor('xo', [RB, DIM], F32, kind='ExternalOutput')
    ld_d = nc.dram_tensor('ld', [RB // NT, NT], F32, kind='ExternalOutput')

    with tile.TileContext(nc) as tc, ExitStack() as ctx:
        masters = ctx.enter_context(tc.tile_pool(name='masters', bufs=1))
        wpool = ctx.enter_context(tc.tile_pool(name='wpool', bufs=2))
        hpool = ctx.enter_context(tc.tile_pool(name='hpool', bufs=2))
        xp = ctx.enter_context(tc.tile_pool(name='xp', bufs=3))
        up = ctx.enter_context(tc.tile_pool(name='up', bufs=2))
        stage = ctx.enter_context(tc.tile_pool(name='stage', bufs=3))
        ostage = ctx.enter_context(tc.tile_pool(name='ostage', bufs=3))
        const = ctx.enter_context(tc.tile_pool(name='const', bufs=1))
        hps = ctx.enter_context(tc.tile_pool(name='hps', bufs=4, space='PSUM'))
        stps = ctx.enter_context(tc.tile_pool(name='stps', bufs=2, space='PSUM'))
        mp = ctx.enter_context(tc.tile_pool(name='mp', bufs=2, space='PSUM'))

        idf = const.tile([128, 128], F32)
        make_identity(nc, idf)
        idb = const.tile([128, 128], BF16)
        nc.vector.tensor_copy(idb, idf)
        fives = const.tile([128, 1], F32)
        nc.vector.memset(fives, S_MAX)
        b1_sb = const.tile([128, NL, HID // 128], F32)
        nc.sync.dma_start(b1_sb, b1_d.ap().rearrange('l (c p) -> p l c', p=128))
        b2_sb = const.tile([128, NL, 2], F32)
        nc.sync.dma_start(b2_sb, b2_d.ap().rearrange('l (c p) -> p l c', p=128))

        for blk in range(NBLK):
            xe = masters.tile([128, NB], F32, tag='xe')
            xom = masters.tile([128, NB], F32, tag='xo')
            cht = masters.tile([128, 2, NB], BF16, tag='ch')
            clt = masters.tile([128, 2, NB], BF16, tag='cl')
            ld_acc = masters.tile([128, NB], F32, tag='ld')

            # ---- load + transpose x (de-interleaved) and context (bf16 pair)
            for t in range(C128):
                rows = blk * NB + t * 128
                xs = stage.tile([128, DIM], F32, tag='xs')
                nc.sync.dma_start(xs, x_d.ap()[ds(rows, 128), :])
                for par, mst in ((0, xe), (1, xom)):
                    pt = mp.tile([128, 128], F32, tag='mp')
                    nc.tensor.transpose(pt, xs[:, par::2], idf[:])
                    nc.vector.tensor_copy(mst[:, ds(t * 128, 128)], pt)
                for src_d, mstc in ((ch_d, cht), (cl_d, clt)):
                    cs = stage.tile([128, CTXF], BF16, tag='cs')
                    nc.sync.dma_start(cs, src_d.ap()[ds(rows, 128), :])
                    for c in range(2):
                        pt2 = mp.tile([128, 128], BF16, tag='mp')
                        nc.tensor.transpose(pt2, cs[:, ds(c * 128, 128)], idb[:])
                        nc.vector.tensor_copy(mstc[:, c, ds(t * 128, 128)], pt2)

            # ---- 6 coupling layers
            # FLOW_REPS>1 repeats the layer loop for timing experiments only
            # (results are numerically meaningless for REPS>1).
            _reps = int(os.environ.get('FLOW_REPS', '1'))
            for L in [l for _ in range(_reps) for l in range(NL)]:
                w1h = wpool.tile([128, 3, HID], BF16, tag='w1h')
                w1l = wpool.tile([128, 3, HID], BF16, tag='w1l')
                w2h = wpool.tile([128, HID // 128, 2 * HALF], BF16, tag='w2h')
                w2l = wpool.tile([128, HID // 128, 2 * HALF], BF16, tag='w2l')
                nc.sync.dma_start(w1h, w1h_d.ap()[L].rearrange('(c p) m -> p c m', p=128))
                nc.sync.dma_start(w1l, w1l_d.ap()[L].rearrange('(c p) m -> p c m', p=128))
                nc.sync.dma_start(w2h, w2h_d.ap()[L].rearrange('(c p) m -> p c m', p=128))
                nc.sync.dma_start(w2l, w2l_d.ap()[L].rearrange('(c p) m -> p c m', p=128))

                xm_m = xe if L % 2 == 0 else xom      # masked features master
                xu_m = xom if L % 2 == 0 else xe      # unmasked (updated)

                for t in range(T_PER_B):
                    tsl = ds(t * NT, NT)
                    xsl = xm_m[:, tsl]
                    xmh = xp.tile([128, NT], BF16, tag='xmh')
                    nc.scalar.copy(xmh, xsl)
                    xml = xp.tile([128, NT], BF16, tag='xml')
                    nc.vector.scalar_tensor_tensor(
                        xml, xsl, 1.0, xmh[:], ALU.mult, ALU.subtract)

                    hhb = hpool.tile([128, HID // 128, NT], BF16, tag='hhb')
                    hlb = hpool.tile([128, HID // 128, NT], BF16, tag='hlb')
                    for m in range(HID // 128):
                        msl = ds(m * 128, 128)
                        hp = hps.tile([128, NT], F32, tag='hp')
                        seq = []
                        for c in range(3):
                            rhs = xmh[:] if c == 0 else cht[:, c - 1, tsl]
                            seq.append((w1h[:, c, msl], rhs))
                        for c in range(3):
                            rhs = xml[:] if c == 0 else clt[:, c - 1, tsl]
                            seq.append((w1h[:, c, msl], rhs))
                        for c in range(3):
                            rhs = xmh[:] if c == 0 else cht[:, c - 1, tsl]
                            seq.append((w1l[:, c, msl], rhs))
                        for i, (lh, rh) in enumerate(seq):
                            nc.tensor.matmul(hp, lh, rh,
                                             start=(i == 0), stop=(i == len(seq) - 1))
                        nc.scalar.activation(hhb[:, m, :], hp, AF.Relu,
                                             bias=b1_sb[:, L, m:m + 1])
                        if b1_zero:
                            nc.vector.scalar_tensor_tensor(
                                hlb[:, m, :], hp, 0.0, hhb[:, m, :],
                                ALU.max, ALU.subtract)
                        else:
                            rt = up.tile([128, NT], F32, tag='rt')
                            nc.vector.tensor_scalar(
                                rt, hp, b1_sb[:, L, m:m + 1], 0.0,
                                ALU.add, ALU.max)
                            nc.vector.tensor_sub(hlb[:, m, :], rt, hhb[:, m, :])

                    s_ps = stps.tile([128, NT], F32, tag='st')
                    t_ps = stps.tile([128, NT], F32, tag='st')
                    for m2, pp in ((0, s_ps), (1, t_ps)):
                        m2sl = ds(m2 * 128, 128)
                        i = 0
                        for wsb, hsb in ((w2h, hhb), (w2h, hlb), (w2l, hhb)):
                            for k in range(HID // 128):
                                nc.tensor.matmul(pp, wsb[:, k, m2sl], hsb[:, k, :],
                                                 start=(i == 0), stop=(i == 23))
                                i += 1
                    u = up.tile([128, NT], F32, tag='u')
                    nc.scalar.activation(u, s_ps, AF.Tanh, bias=b2_sb[:, L, 0:1])
                    e = up.tile([128, NT], F32, tag='e')
                    nc.scalar.activation(e, u, AF.Exp, scale=S_MAX)
                    if L == 0:
                        nc.vector.tensor_copy(ld_acc[:, tsl], u)
                    else:
                        nc.vector.tensor_add(ld_acc[:, tsl], ld_acc[:, tsl], u)
                    xus = xu_m[:, tsl]
                    tmp = up.tile([128, NT], F32, tag='tmp')
                    nc.vector.tensor_mul(tmp, xus, e)
                    nc.vector.scalar_tensor_tensor(
                        xus, t_ps, b2_sb[:, L, 1:2], tmp[:], ALU.add, ALU.add)

            # ---- log_det fold (x5 via fives vector) and writeback transposes
            for t in range(T_PER_B):
                lp = mp.tile([1, NT], F32, tag='mp')
                nc.tensor.matmul(lp, fives[:], ld_acc[:, ds(t * NT, NT)],
                                 start=True, stop=True)
                lst = ostage.tile([1, NT], F32, tag='lst')
                nc.scalar.copy(lst, lp)
                nc.sync.dma_start(ld_d.ap()[blk * T_PER_B + t:blk * T_PER_B + t + 1, :], lst)

            for t in range(C128):
                rows = blk * NB + t * 128
                ot = ostage.tile([128, DIM], F32, tag='ot')
                for par, mst in ((0, xe), (1, xom)):
                    pt = mp.tile([128, 128], F32, tag='mp')
                    nc.tensor.transpose(pt, mst[:, ds(t * 128, 128)], idf[:])
                    nc.vector.tensor_copy(ot[:, par::2], pt)
                nc.sync.dma_start(xo_d.ap()[ds(rows, 128), :], ot)

    nc.compile()
    return nc


_CACHE = {}


def _get_runner(b1_zero):
    """Build (once) and return a reusable jitted 8-core runner."""
    key = ('split3', b1_zero)
    if key in _CACHE:
        return _CACHE[key]

    import jax
    from jax.sharding import Mesh, PartitionSpec
    from jax.experimental.shard_map import shard_map
    from concourse import bass2jax

    nc = _build_program(b1_zero=b1_zero)
    bass2jax.install_neuronx_cc_hook()

    partition_name = (nc.partition_id_tensor.name
                      if nc.partition_id_tensor else None)
    in_names = []
    out_names = []
    out_avals = []
    out_shapes = []
    for alloc in nc.m.functions[0].allocations:
        if not isinstance(alloc, mybir.MemoryLocationSet):
            continue
        name = alloc.memorylocations[0].name
        if alloc.kind == 'ExternalInput':
            if name != partition_name:
                in_names.append(name)
        elif alloc.kind == 'ExternalOutput':
            out_names.append(name)
            shape = tuple(alloc.tensor_shape)
            npdt = mybir.dt.np(alloc.dtype)
            out_avals.append(jax.core.ShapedArray(shape, npdt))
            out_shapes.append((shape, npdt))
    n_params = len(in_names)
    all_names = in_names + out_names
    if partition_name is not None:
        all_names = all_names + [partition_name]

    def _body(*args):
        operands = list(args)
        if partition_name is not None:
            operands.append(bass2jax.partition_id_tensor())
        outs = bass2jax._bass_exec_p.bind(
            *operands,
            out_avals=tuple(out_avals),
            in_names=tuple(all_names),
            out_names=tuple(out_names),
            lowering_input_output_aliases=(),
            sim_require_finite=True,
            sim_require_nnan=True,
            nc=nc,
        )
        return tuple(outs)

    devices = jax.devices()[:NCORES]
    mesh = Mesh(np.asarray(devices), ('core',))
    nin = n_params + len(out_names)
    sharded = jax.jit(shard_map(
        _body, mesh=mesh,
        in_specs=(PartitionSpec('core'),) * nin,
        out_specs=(PartitionSpec('core'),) * len(out_names),
        check_rep=False))

    runner = {
        'nc': nc, 'sharded': sharded, 'in_names': in_names,
        'out_names': out_names, 'out_shapes': out_shapes, 'jax': jax,
    }
    _CACHE[key] = runner
    return runner


def _prepare_inputs(x, context, W1, b1, W2, b2):
    """Host-side shard + split. Returns dict name -> global (8*d0, ...) array."""
    bf = ml_dtypes.bfloat16
    x = np.ascontiguousarray(np.asarray(x, np.float32))
    context = np.ascontiguousarray(np.asarray(context, np.float32))
    W1 = np.asarray(W1, np.float32)
    b1 = np.asarray(b1, np.float32)
    W2 = np.asarray(W2, np.float32)
    b2 = np.asarray(b2, np.float32)

    ctxh = context.astype(bf)
    ctxl = (context - ctxh.astype(np.float32)).astype(bf)
    w1h = W1.astype(bf)
    w1l = (W1 - w1h.astype(np.float32)).astype(bf)
    w2h = W2.astype(bf)
    w2l = (W2 - w2h.astype(np.float32)).astype(bf)

    reps = {
        'w1h': w1h, 'w1l': w1l, 'w2h': w2h, 'w2l': w2l,
        'b1': b1, 'b2': b2,
    }
    glob = {'x': x, 'ctxh': ctxh, 'ctxl': ctxl}
    out = dict(glob)
    # replicated tensors: tile along axis 0 for the shard_map concat layout
    for k, v in reps.items():
        out[k] = np.concatenate([v] * NCORES, axis=0)
    return out


def _run(runner, host_inputs, time_reps=0):
    jax = runner['jax']
    zeros = [np.zeros((NCORES * s[0],) + tuple(s[1:]), d)
             for (s, d) in runner['out_shapes']]
    args = [host_inputs[n] for n in runner['in_names']] + zeros
    dev_args = [jax.device_put(a) for a in args]
    outs = runner['sharded'](*dev_args)
    outs = [np.asarray(o) for o in outs]
    res = dict(zip(runner['out_names'], outs))

    timing = None
    if time_reps:
        import time as _t
        for _ in range(1):  # warm
            r = runner['sharded'](*dev_args)
            jax.block_until_ready(r)
        ts = []
        for _ in range(time_reps):
            t0 = _t.perf_counter()
            r = runner['sharded'](*dev_args)
            jax.block_until_ready(r)
            ts.append(_t.perf_counter() - t0)
        timing = min(ts)
    return res, timing


def kernel(x, context, W1, b1, W2, b2, _time_reps=0):
    b1_zero = not np.any(np.asarray(b1))
    runner = _get_runner(b1_zero)
    host_inputs = _prepare_inputs(x, context, W1, b1, W2, b2)
    res, timing = _run(runner, host_inputs, time_reps=_time_reps)

    x_out = res['xo'].reshape(B, DIM).astype(np.float32, copy=False)
    ld = res['ld'].reshape(B).astype(np.float32, copy=False)
    if _time_reps:
        kernel.last_timing = timing
    return x_out, ld


if __name__ == '__main__':
    rng = np.random.default_rng(0)
    ins = {
        'x': rng.standard_normal((B, DIM)).astype(np.float32),
        'context': rng.standard_normal((B, CTXF)).astype(np.float32),
        'W1': (rng.standard_normal((NL, HALF + CTXF, HID)) * 0.02).astype(np.float32),
        'b1': np.zeros((NL, HID), np.float32),
        'W2': (rng.standard_normal((NL, HID, 2 * HALF)) * 0.02).astype(np.float32),
        'b2': np.zeros((NL, 2 * HALF), np.float32),
    }
    xo, ld = kernel(**ins)
    print('ran ok', xo.shape, ld.shape, float(np.abs(xo).max()), float(np.abs(ld).max()))


# revision 7
# speedup vs baseline: 353.6271x; 353.6271x over previous
"""Trainium2 Bass kernel for a 6-layer RealNVP-style conditional flow.

kernel(x, context, W1, b1, W2, b2) -> (x_out [65536,256] f32, log_det [65536] f32)

Strategy:
  - Pure data parallelism: batch sharded 8192 rows/core across 8 NeuronCores,
    weights replicated.
  - Activations kept transposed [feature, batch] in SBUF; x stored as
    de-interleaved even/odd feature masters so each coupling layer's
    masked/unmasked halves are contiguous 128-partition tiles.
  - All matmuls in split-bf16 3-product form (Xh@Wh + Xl@Wh + Xh@Wl with
    fp32 PSUM accumulation) for ~22-bit effective mantissa at 1 cycle/row.
    Weights and context are split on the host; x and h are split on device.
  - log_det accumulated as sum of tanh(st) in SBUF, folded over features by a
    single [128,1] fives-vector matmul per 512-column tile (x5 scale baked in).
"""

import os
import sys
from contextlib import ExitStack

import numpy as np

for _p in ('/opt/trn_rl_repo', '/root/.axon_site/_ro/trn_rl_repo'):
    if os.path.isdir(_p) and _p not in sys.path:
        sys.path.insert(0, _p)

import ml_dtypes  # noqa: E402
import concourse.bass as bass  # noqa: E402
import concourse.tile as tile  # noqa: E402
from concourse import bacc, mybir  # noqa: E402
from concourse.bass import ds  # noqa: E402
from concourse.masks import make_identity  # noqa: E402

dt = mybir.dt
F32 = dt.float32
F32R = dt.float32r
BF16 = dt.bfloat16
AF = mybir.ActivationFunctionType
ALU = mybir.AluOpType

NCORES = 8
B = 65536
RB = B // NCORES      # 8192 rows per core
DIM = 256
HALF = 128
CTXF = 256
HID = 1024
NL = 6
S_MAX = 5.0

NB = 4096             # batch columns per block (2 blocks per core)
NT = 512              # batch-tile columns (matmul moving dim)
NBLK = RB // NB       # 2
T_PER_B = NB // NT    # 8
C128 = NB // 128      # 32 column-tiles of 128 for transposes


def _build_program(b1_zero=True):
    nc = bacc.Bacc('TRN2', target_bir_lowering=False, debug=False,
                   num_devices=NCORES)

    x_d = nc.dram_tensor('x', [RB, DIM], F32, kind='ExternalInput')
    ch_d = nc.dram_tensor('ctxh', [RB, CTXF], BF16, kind='ExternalInput')
    cl_d = nc.dram_tensor('ctxl', [RB, CTXF], BF16, kind='ExternalInput')
    w1h_d = nc.dram_tensor('w1h', [NL, HALF + CTXF, HID], BF16, kind='ExternalInput')
    w1l_d = nc.dram_tensor('w1l', [NL, HALF + CTXF, HID], BF16, kind='ExternalInput')
    w2h_d = nc.dram_tensor('w2h', [NL, HID, 2 * HALF], BF16, kind='ExternalInput')
    w2l_d = nc.dram_tensor('w2l', [NL, HID, 2 * HALF], BF16, kind='ExternalInput')
    b1_d = nc.dram_tensor('b1', [NL, HID], F32, kind='ExternalInput')
    b2_d = nc.dram_tensor('b2', [NL, 2 * HALF], F32, kind='ExternalInput')
    xo_d = nc.dram_tensor('xo', [RB, DIM], F32, kind='ExternalOutput')
    ld_d = nc.dram_tensor('ld', [RB // NT, NT], F32, kind='ExternalOutput')

    with tile.TileContext(nc) as tc, ExitStack() as ctx:
        masters = ctx.enter_context(tc.tile_pool(name='masters', bufs=1))
        wpool = ctx.enter_context(tc.tile_pool(name='wpool', bufs=2))
        hpool = ctx.enter_context(tc.tile_pool(name='hpool', bufs=2))
        xp = ctx.enter_context(tc.tile_pool(name='xp', bufs=3))
        up = ctx.enter_context(tc.tile_pool(name='up', bufs=2))
        stage = ctx.enter_context(tc.tile_pool(name='stage', bufs=3))
        ostage = ctx.enter_context(tc.tile_pool(name='ostage', bufs=3))
        const = ctx.enter_context(tc.tile_pool(name='const', bufs=1))
        hps = ctx.enter_context(tc.tile_pool(name='hps', bufs=4, space='PSUM'))
        stps = ctx.enter_context(tc.tile_pool(name='stps', bufs=2, space='PSUM'))
        mp = ctx.enter_context(tc.tile_pool(name='mp', bufs=2, space='PSUM'))

        idf = const.tile([128, 128], F32)
        make_identity(nc, idf)
        idb = const.tile([128, 128], BF16)
        nc.vector.tensor_copy(idb, idf)
        fives = const.tile([128, 1], F32)
        nc.vector.memset(fives, S_MAX)
        b1_sb = const.tile([128, NL, HID // 128], F32)
        nc.sync.dma_start(b1_sb, b1_d.ap().rearrange('l (c p) -> p l c', p=128))
        b2_sb = const.tile([128, NL, 2], F32)
        nc.sync.dma_start(b2_sb, b2_d.ap().rearrange('l (c p) -> p l c', p=128))

        for blk in range(NBLK):
            xe = masters.tile([128, NB], F32, tag='xe')
            xom = masters.tile([128, NB], F32, tag='xo')
            cht = masters.tile([128, 2, NB], BF16, tag='ch')
            clt = masters.tile([128, 2, NB], BF16, tag='cl')
            ld_acc = masters.tile([128, NB], F32, tag='ld')

            # ---- load + transpose x (de-interleaved) and context (bf16 pair)
            for t in range(C128):
                rows = blk * NB + t * 128
                xs = stage.tile([128, DIM], F32, tag='xs')
                nc.sync.dma_start(xs, x_d.ap()[ds(rows, 128), :])
                for par, mst in ((0, xe), (1, xom)):
                    pt = mp.tile([128, 128], F32, tag='mp')
                    nc.tensor.transpose(pt, xs[:, par::2], idf[:])
                    nc.vector.tensor_copy(mst[:, ds(t * 128, 128)], pt)
                for src_d, mstc in ((ch_d, cht), (cl_d, clt)):
                    cs = stage.tile([128, CTXF], BF16, tag='cs')
                    nc.sync.dma_start(cs, src_d.ap()[ds(rows, 128), :])
                    for c in range(2):
                        pt2 = mp.tile([128, 128], BF16, tag='mp')
                        nc.tensor.transpose(pt2, cs[:, ds(c * 128, 128)], idb[:])
                        nc.vector.tensor_copy(mstc[:, c, ds(t * 128, 128)], pt2)

            # ---- 6 coupling layers
            # FLOW_REPS>1 repeats the layer loop for timing experiments only
            # (results are numerically meaningless for REPS>1).
            _reps = int(os.environ.get('FLOW_REPS', '1'))
            for L in [l for _ in range(_reps) for l in range(NL)]:
                w1h = wpool.tile([128, 3, HID], BF16, tag='w1h')
                w1l = wpool.tile([128, 3, HID], BF16, tag='w1l')
                w2h = wpool.tile([128, HID // 128, 2 * HALF], BF16, tag='w2h')
                w2l = wpool.tile([128, HID // 128, 2 * HALF], BF16, tag='w2l')
                nc.sync.dma_start(w1h, w1h_d.ap()[L].rearrange('(c p) m -> p c m', p=128))
                nc.sync.dma_start(w1l, w1l_d.ap()[L].rearrange('(c p) m -> p c m', p=128))
                nc.sync.dma_start(w2h, w2h_d.ap()[L].rearrange('(c p) m -> p c m', p=128))
                nc.sync.dma_start(w2l, w2l_d.ap()[L].rearrange('(c p) m -> p c m', p=128))

                xm_m = xe if L % 2 == 0 else xom      # masked features master
                xu_m = xom if L % 2 == 0 else xe      # unmasked (updated)

                for t in range(T_PER_B):
                    tsl = ds(t * NT, NT)
                    xsl = xm_m[:, tsl]
                    xmh = xp.tile([128, NT], BF16, tag='xmh')
                    nc.scalar.copy(xmh, xsl)
                    xml = xp.tile([128, NT], BF16, tag='xml')
                    nc.vector.scalar_tensor_tensor(
                        xml, xsl, 1.0, xmh[:], ALU.mult, ALU.subtract)

                    hhb = hpool.tile([128, HID // 128, NT], BF16, tag='hhb')
                    hlb = hpool.tile([128, HID // 128, NT], BF16, tag='hlb')
                    for m in range(HID // 128):
                        msl = ds(m * 128, 128)
                        hp = hps.tile([128, NT], F32, tag='hp')
                        seq = []
                        for c in range(3):
                            rhs = xmh[:] if c == 0 else cht[:, c - 1, tsl]
                            seq.append((w1h[:, c, msl], rhs))
                        for c in range(3):
                            rhs = xml[:] if c == 0 else clt[:, c - 1, tsl]
                            seq.append((w1h[:, c, msl], rhs))
                        for c in range(3):
                            rhs = xmh[:] if c == 0 else cht[:, c - 1, tsl]
                            seq.append((w1l[:, c, msl], rhs))
                        for i, (lh, rh) in enumerate(seq):
                            nc.tensor.matmul(hp, lh, rh,
                                             start=(i == 0), stop=(i == len(seq) - 1))
                        nc.scalar.activation(hhb[:, m, :], hp, AF.Relu,
                                             bias=b1_sb[:, L, m:m + 1])
                        if b1_zero:
                            nc.vector.scalar_tensor_tensor(
                                hlb[:, m, :], hp, 0.0, hhb[:, m, :],
                                ALU.max, ALU.subtract)
                        else:
                            rt = up.tile([128, NT], F32, tag='rt')
                            nc.vector.tensor_scalar(
                                rt, hp, b1_sb[:, L, m:m + 1], 0.0,
                                ALU.add, ALU.max)
                            nc.vector.tensor_sub(hlb[:, m, :], rt, hhb[:, m, :])

                    s_ps = stps.tile([128, NT], F32, tag='st')
                    t_ps = stps.tile([128, NT], F32, tag='st')
                    for m2, pp in ((0, s_ps), (1, t_ps)):
                        m2sl = ds(m2 * 128, 128)
                        i = 0
                        for wsb, hsb in ((w2h, hhb), (w2h, hlb), (w2l, hhb)):
                            for k in range(HID // 128):
                                nc.tensor.matmul(pp, wsb[:, k, m2sl], hsb[:, k, :],
                                                 start=(i == 0), stop=(i == 23))
                                i += 1
                    u = up.tile([128, NT], F32, tag='u')
                    nc.scalar.activation(u, s_ps, AF.Tanh, bias=b2_sb[:, L, 0:1])
                    e = up.tile([128, NT], F32, tag='e')
                    nc.scalar.activation(e, u, AF.Exp, scale=S_MAX)
                    if L == 0:
                        nc.vector.tensor_copy(ld_acc[:, tsl], u)
                    else:
                        nc.vector.tensor_add(ld_acc[:, tsl], ld_acc[:, tsl], u)
                    xus = xu_m[:, tsl]
                    tmp = up.tile([128, NT], F32, tag='tmp')
                    nc.vector.tensor_mul(tmp, xus, e)
                    nc.vector.scalar_tensor_tensor(
                        xus, t_ps, b2_sb[:, L, 1:2], tmp[:], ALU.add, ALU.add)

            # ---- log_det fold (x5 via fives vector) and writeback transposes
            for t in range(T_PER_B):
                lp = mp.tile([1, NT], F32, tag='mp')
                nc.tensor.matmul(lp, fives[:], ld_acc[:, ds(t * NT, NT)],
                                 start=True, stop=True)
                lst = ostage.tile([1, NT], F32, tag='lst')
                nc.scalar.copy(lst, lp)
                nc.sync.dma_start(ld_d.ap()[blk * T_PER_B + t:blk * T_PER_B + t + 1, :], lst)

            for t in range(C128):
                rows = blk * NB + t * 128
                ot = ostage.tile([128, DIM], F32, tag='ot')
                for par, mst in ((0, xe), (1, xom)):
                    pt = mp.tile([128, 128], F32, tag='mp')
                    nc.tensor.transpose(pt, mst[:, ds(t * 128, 128)], idf[:])
                    nc.vector.tensor_copy(ot[:, par::2], pt)
                nc.sync.dma_start(xo_d.ap()[ds(rows, 128), :], ot)

    nc.compile()
    return nc


_CACHE = {}


def _get_runner(b1_zero):
    """Build (once) and return a reusable jitted 8-core runner."""
    key = ('split3', b1_zero)
    if key in _CACHE:
        return _CACHE[key]

    import jax
    from jax.sharding import Mesh, PartitionSpec
    from jax.experimental.shard_map import shard_map
    from concourse import bass2jax

    nc = _build_program(b1_zero=b1_zero)
    bass2jax.install_neuronx_cc_hook()

    partition_name = (nc.partition_id_tensor.name
                      if nc.partition_id_tensor else None)
    in_names = []
    out_names = []
    out_avals = []
    out_shapes = []
    for alloc in nc.m.functions[0].allocations:
        if not isinstance(alloc, mybir.MemoryLocationSet):
            continue
        name = alloc.memorylocations[0].name
        if alloc.kind == 'ExternalInput':
            if name != partition_name:
                in_names.append(name)
        elif alloc.kind == 'ExternalOutput':
            out_names.append(name)
            shape = tuple(alloc.tensor_shape)
            npdt = mybir.dt.np(alloc.dtype)
            out_avals.append(jax.core.ShapedArray(shape, npdt))
            out_shapes.append((shape, npdt))
    n_params = len(in_names)
    all_names = in_names + out_names
    if partition_name is not None:
        all_names = all_names + [partition_name]

    def _body(*args):
        operands = list(args)
        if partition_name is not None:
            operands.append(bass2jax.partition_id_tensor())
        outs = bass2jax._bass_exec_p.bind(
            *operands,
            out_avals=tuple(out_avals),
            in_names=tuple(all_names),
            out_names=tuple(out_names),
            lowering_input_output_aliases=(),
            sim_require_finite=True,
            sim_require_nnan=True,
            nc=nc,
        )
        return tuple(outs)

    devices = jax.devices()[:NCORES]
    mesh = Mesh(np.asarray(devices), ('core',))
    nin = n_params + len(out_names)
    sharded = jax.jit(shard_map(
        _body, mesh=mesh,
        in_specs=(PartitionSpec('core'),) * nin,
        out_specs=(PartitionSpec('core'),) * len(out_names),
        check_rep=False))

    runner = {
        'nc': nc, 'sharded': sharded, 'in_names': in_names,
        'out_names': out_names, 'out_shapes': out_shapes, 'jax': jax,
        'mesh': mesh,
        'sharding': jax.sharding.NamedSharding(mesh, PartitionSpec('core')),
    }
    _CACHE[key] = runner
    return runner


def _prepare_inputs(x, context, W1, b1, W2, b2):
    """Host-side shard + split. Returns dict name -> global (8*d0, ...) array."""
    bf = ml_dtypes.bfloat16
    x = np.ascontiguousarray(np.asarray(x, np.float32))
    context = np.ascontiguousarray(np.asarray(context, np.float32))
    W1 = np.asarray(W1, np.float32)
    b1 = np.asarray(b1, np.float32)
    W2 = np.asarray(W2, np.float32)
    b2 = np.asarray(b2, np.float32)

    ctxh = context.astype(bf)
    ctxl = (context - ctxh.astype(np.float32)).astype(bf)
    w1h = W1.astype(bf)
    w1l = (W1 - w1h.astype(np.float32)).astype(bf)
    w2h = W2.astype(bf)
    w2l = (W2 - w2h.astype(np.float32)).astype(bf)

    reps = {
        'w1h': w1h, 'w1l': w1l, 'w2h': w2h, 'w2l': w2l,
        'b1': b1, 'b2': b2,
    }
    glob = {'x': x, 'ctxh': ctxh, 'ctxl': ctxl}
    out = dict(glob)
    # replicated tensors: tile along axis 0 for the shard_map concat layout
    for k, v in reps.items():
        out[k] = np.concatenate([v] * NCORES, axis=0)
    return out


def _run(runner, host_inputs, time_reps=0):
    jax = runner['jax']
    zeros = [np.zeros((NCORES * s[0],) + tuple(s[1:]), d)
             for (s, d) in runner['out_shapes']]
    args = [host_inputs[n] for n in runner['in_names']] + zeros
    dev_args = [jax.device_put(a, runner['sharding']) for a in args]
    outs = runner['sharded'](*dev_args)
    outs = [np.asarray(o) for o in outs]
    res = dict(zip(runner['out_names'], outs))

    timing = None
    if time_reps:
        import time as _t
        for _ in range(1):  # warm
            r = runner['sharded'](*dev_args)
            jax.block_until_ready(r)
        ts = []
        for _ in range(time_reps):
            t0 = _t.perf_counter()
            r = runner['sharded'](*dev_args)
            jax.block_until_ready(r)
            ts.append(_t.perf_counter() - t0)
        timing = min(ts)
    return res, timing


def kernel(x, context, W1, b1, W2, b2, _time_reps=0):
    b1_zero = not np.any(np.asarray(b1))
    runner = _get_runner(b1_zero)
    host_inputs = _prepare_inputs(x, context, W1, b1, W2, b2)
    res, timing = _run(runner, host_inputs, time_reps=_time_reps)

    x_out = res['xo'].reshape(B, DIM).astype(np.float32, copy=False)
    ld = res['ld'].reshape(B).astype(np.float32, copy=False)
    if _time_reps:
        kernel.last_timing = timing
    return x_out, ld


if __name__ == '__main__':
    rng = np.random.default_rng(0)
    ins = {
        'x': rng.standard_normal((B, DIM)).astype(np.float32),
        'context': rng.standard_normal((B, CTXF)).astype(np.float32),
        'W1': (rng.standard_normal((NL, HALF + CTXF, HID)) * 0.02).astype(np.float32),
        'b1': np.zeros((NL, HID), np.float32),
        'W2': (rng.standard_normal((NL, HID, 2 * HALF)) * 0.02).astype(np.float32),
        'b2': np.zeros((NL, 2 * HALF), np.float32),
    }
    xo, ld = kernel(**ins)
    print('ran ok', xo.shape, ld.shape, float(np.abs(xo).max()), float(np.abs(ld).max()))
